# revision 1
# baseline (speedup 1.0000x reference)
"""Trainium2 Bass kernel for nn_Attention4D_77644418777285.

Attention4D block (EfficientViT-style): 1x1-conv QKV + BN, depthwise-3x3
local-V branch, relative-position bias, talking-heads attention (8 heads,
49 tokens), projection. Batch 512 sharded 64-per-core across 8 NeuronCores
(pure data parallel; weights replicated).

Strategy (per core, 64 images = 3136 tokens):
  - x transposed on PE to channel-major xT [384, 3136] (bf16).
  - QKV as channel-major matmuls (weights stationary), BN folded into
    weights/biases on host; softmax scale folded into q weights.
  - Attention middle processed in groups of 8 images with logits held as
    [(head-pair, m) x (img, n)] tiles: per-(img,head) qk matmuls, talking
    heads th1/th2 as constant 128x128 block matmuls, softmax (exp on ACT,
    column sums via a constant selector matmul, reciprocal on DVE,
    normalization broadcast via constant delta matmul).
  - v also computed token-major per image pair (separate matmul) for the
    attention*V product; output is channel-major o^T per head.
  - depthwise 3x3 conv on DVE: 9 fused scalar_tensor_tensor taps over a
    zero-padded 9x9 channel-major layout; per-channel tap weights native as
    [P,1] scalars. All conv/BN biases folded into a precomputed B2 term
    that seeds the accumulator via DMA.
  - o^T + v_local + relu, projection back to channel-major, PE transpose to
    token-major f32 output.
"""

import numpy as np
import ml_dtypes

R = 7
N = 49
H = 8
KD = 32
D = 128
DH = 1024
DIM = 384
SCALE = KD ** -0.5
NCORES = 8
B_FULL = 512

PAD = 81           # 9x9 padded spatial slots per image (channel-major v)
GUARD = 16         # zero guard columns at each end of the padded v tiles

_BF16 = ml_dtypes.bfloat16


def _bias_idxs(r):
    pos = np.stack(np.meshgrid(np.arange(r), np.arange(r))).reshape(2, -1)
    rel = np.abs(pos[:, :, None] - pos[:, None, :])
    return (rel[0] * r + rel[1]).reshape(-1)


def _host_consts(inp):
    """All weight-shaped tensors precomputed on host (numpy)."""
    f32 = np.float32
    g = {k: np.asarray(v, f32) for k, v in inp.items()}

    th1, th1_b = g['th1_w'], g['th1_b']
    th2, th2_b = g['th2_w'], g['th2_b']

    W_q = g['q_w'] * g['q_g'][None, :] * SCALE
    b_q = (g['q_b'] * g['q_g'] + g['q_beta']) * SCALE
    W_k = g['k_w'] * g['k_g'][None, :]
    b_k = g['k_b'] * g['k_g'] + g['k_beta']
    W_v = g['v_w'] * g['v_g'][None, :]
    b_v = g['v_b'] * g['v_g'] + g['v_beta']

    idxs = _bias_idxs(R)
    bias_full = g['attn_bias'][:, idxs].reshape(H, N, N)          # [h, n, m]
    biasp = np.einsum('hg,hnm->gnm', th1, bias_full) + th1_b[:, None, None]

    w9 = g['vl_w'].reshape(9, DH)                                  # [tap, c]
    w_eff = (w9 * g['vl_g'][None, :]).astype(f32)                  # [tap, c]
    sumw = np.zeros((DH, N), f32)
    for t in range(9):
        dy, dx = t // 3 - 1, t % 3 - 1
        for s in range(N):
            y, x = s // 7, s % 7
            if 0 <= y + dy < 7 and 0 <= x + dx < 7:
                sumw[:, s] += w9[t]
    s2 = th2.sum(axis=0) + N * th2_b                               # [g]
    B2 = (g['vl_g'][:, None] * (b_v[:, None] * sumw + g['vl_b'][:, None])
          + g['vl_beta'][:, None]
          + (b_v * s2[np.repeat(np.arange(H), D)])[:, None])       # [c, s=49]

    W_p = g['proj_w'] * g['proj_g'][None, :]
    b_p = g['proj_b'] * g['proj_g'] + g['proj_beta']

    consts = {}
    # QKV weights: [3 ktiles, 128, 512] (q|k) and [3, 128, 1024] (v)
    wqk = np.concatenate([W_q, W_k], axis=1).reshape(3, 128, 512)
    consts['wqk'] = wqk.astype(_BF16)
    consts['wv'] = W_v.reshape(3, 128, DH).astype(_BF16)
    consts['wp'] = W_p.reshape(8, 128, DIM).astype(_BF16)
    consts['bqk'] = np.concatenate([b_q, b_k]).reshape(4, 128).astype(f32)
    consts['bp'] = b_p.reshape(3, 128).astype(f32)

    # Talking heads as [jo, ji, K=128, M=128] block matrices in the 0/64
    # partition layout: row (hh*64 + m) of input tile ji = head (2*ji+hh),
    # key m; col (hh*64 + m) of output tile jo = head (2*jo+hh), key m.
    def th_blocks(thw):
        Wb = np.zeros((4, 4, 128, 128), f32)
        for jo in range(4):
            for ji in range(4):
                for hhi in range(2):
                    for hho in range(2):
                        c = thw[2 * ji + hhi, 2 * jo + hho]
                        Wb[jo, ji, hhi * 64:hhi * 64 + N,
                           hho * 64:hho * 64 + N] += c * np.eye(N, dtype=f32)
        return Wb
    consts['w1s'] = th_blocks(th1).astype(_BF16)
    consts['w2s'] = th_blocks(th2).astype(_BF16)

    sel = np.zeros((128, 2), f32)
    sel[0:N, 0] = 1.0
    sel[64:64 + N, 1] = 1.0
    consts['sel'] = sel.astype(_BF16)

    dlt = np.zeros((128, 128), f32)
    for j in range(4):
        dlt[32 * j + 0, 0:N] = 1.0
        dlt[32 * j + 1, 64:64 + N] = 1.0
    consts['dlt'] = dlt.astype(f32)

    # exp(bias') in the [(hh,m) x (img, n)] layout, replicated over 8 images,
    # zeros in dead rows (49-63, 113-127).
    expb = np.zeros((4, 128, 8 * N), f32)
    for j in range(4):
        for hh in range(2):
            e = np.exp(biasp[2 * j + hh].T)                        # [m, n]
            expb[j, hh * 64:hh * 64 + N] = np.tile(e, (1, 8))
    consts['expb'] = expb.astype(_BF16)

    # depthwise tap weights: sbuf [128, 8*9] (c-part, (ct, tap))
    w9t = w_eff.reshape(9, 8, 128).transpose(2, 1, 0)              # [c128? no]
    # w_eff[tap, c]: c = ct*128 + p -> dst [p, ct, tap]
    w9t = w_eff.reshape(9, 8, 128).transpose(2, 1, 0).copy()       # [128, 8, 9]
    consts['w9t'] = w9t.astype(f32)

    # B2 in padded 81-slot layout, zeros at pad slots: [8, 128, 81]
    b2p = np.zeros((8, 128, PAD), f32)
    b2v = B2.reshape(8, 128, 7, 7)
    for y in range(7):
        for x in range(7):
            b2p[:, :, (y + 1) * 9 + (x + 1)] = b2v[:, :, y, x]
    consts['b2p'] = b2p.astype(_BF16)

    consts['ident'] = np.eye(128, dtype=f32).astype(_BF16)
    return consts


def build_program(n_imgs, stage=99):
    """Build the Bass program for one core processing n_imgs images."""
    from contextlib import ExitStack
    import concourse.bass as bass
    import concourse.tile as tile
    from concourse import bacc, mybir

    f32 = mybir.dt.float32
    bf16 = mybir.dt.bfloat16
    AF = mybir.ActivationFunctionType
    ALU = mybir.AluOpType

    NI = n_imgs
    NG = NI // 8                 # groups of 8 images
    NT = NI * N                  # tokens
    NTT = (NT + 127) // 128      # token tiles
    GP = 8 * PAD                 # padded cols per group in v_cm

    nc = bacc.Bacc("TRN2", target_bir_lowering=False, debug=False,
                   enable_asserts=False)

    x_d = nc.dram_tensor("x", [NT, DIM], bf16, kind="ExternalInput").ap()
    wqk_d = nc.dram_tensor("wqk", [3, 128, 512], bf16, kind="ExternalInput").ap()
    wv_d = nc.dram_tensor("wv", [3, 128, DH], bf16, kind="ExternalInput").ap()
    wp_d = nc.dram_tensor("wp", [8, 128, DIM], bf16, kind="ExternalInput").ap()
    bqk_d = nc.dram_tensor("bqk", [4, 128], f32, kind="ExternalInput").ap()
    bp_d = nc.dram_tensor("bp", [3, 128], f32, kind="ExternalInput").ap()
    w1_d = nc.dram_tensor("w1s", [4, 4, 128, 128], bf16, kind="ExternalInput").ap()
    w2_d = nc.dram_tensor("w2s", [4, 4, 128, 128], bf16, kind="ExternalInput").ap()
    sel_d = nc.dram_tensor("sel", [128, 2], bf16, kind="ExternalInput").ap()
    dlt_d = nc.dram_tensor("dlt", [128, 128], f32, kind="ExternalInput").ap()
    expb_d = nc.dram_tensor("expb", [4, 128, 392], bf16, kind="ExternalInput").ap()
    w9_d = nc.dram_tensor("w9t", [128, 8, 9], f32, kind="ExternalInput").ap()
    b2_d = nc.dram_tensor("b2p", [8, 128, PAD], bf16, kind="ExternalInput").ap()
    id_d = nc.dram_tensor("ident", [128, 128], bf16, kind="ExternalInput").ap()
    out_d = nc.dram_tensor("out", [NT, DIM], f32, kind="ExternalOutput").ap()

    with tile.TileContext(nc) as tc, ExitStack() as ctx:
        const = ctx.enter_context(tc.tile_pool(name="const", bufs=1))
        pers = ctx.enter_context(tc.tile_pool(name="pers", bufs=1))
        xin = ctx.enter_context(tc.tile_pool(name="xin", bufs=2))
        mid = ctx.enter_context(tc.tile_pool(name="mid", bufs=6))
        accp = ctx.enter_context(tc.tile_pool(name="accp", bufs=1))
        stg = ctx.enter_context(tc.tile_pool(name="stg", bufs=2))
        ps = ctx.enter_context(tc.tile_pool(name="ps", bufs=7, space="PSUM"))

        dma = nc.sync.dma_start

        # ---------------- constants ----------------
        wqk_t = [const.tile([128, 512], bf16, name=f"wqk{k}", tag=f"wqk{k}") for k in range(3)]
        wv_t = [const.tile([128, DH], bf16, name=f"wv{k}", tag=f"wv{k}") for k in range(3)]
        wp_t = [const.tile([128, DIM], bf16, name=f"wp{k}", tag=f"wp{k}") for k in range(8)]
        for k in range(3):
            dma(out=wqk_t[k], in_=wqk_d[k])
            dma(out=wv_t[k], in_=wv_d[k])
        for k in range(8):
            dma(out=wp_t[k], in_=wp_d[k])
        bqk_t = const.tile([128, 4], f32, name="bqk", tag="bqk")
        dma(out=bqk_t, in_=bass.AP(tensor=bqk_d.tensor, offset=0,
                                   ap=[[1, 128], [128, 4]]))
        bp_t = const.tile([128, 3], f32, name="bp", tag="bp")
        dma(out=bp_t, in_=bass.AP(tensor=bp_d.tensor, offset=0,
                                  ap=[[1, 128], [128, 3]]))
        w1_t = const.tile([128, 16, 128], bf16, name="w1", tag="w1")
        dma(out=w1_t, in_=bass.AP(tensor=w1_d.tensor, offset=0,
                                  ap=[[128, 128], [128 * 128, 16], [1, 128]]))
        w2_t = const.tile([128, 16, 128], bf16, name="w2", tag="w2")
        dma(out=w2_t, in_=bass.AP(tensor=w2_d.tensor, offset=0,
                                  ap=[[128, 128], [128 * 128, 16], [1, 128]]))
        sel_t = const.tile([128, 2], bf16, name="sel", tag="sel")
        dma(out=sel_t, in_=sel_d)
        dlt_t = const.tile([128, 128], f32, name="dlt", tag="dlt")
        dma(out=dlt_t, in_=dlt_d)
        expb_t = [const.tile([128, 392], bf16, name=f"eb{j}", tag=f"eb{j}") for j in range(4)]
        for j in range(4):
            dma(out=expb_t[j], in_=expb_d[j])
        w9_t = const.tile([128, 8, 9], f32, name="w9", tag="w9")
        dma(out=w9_t, in_=w9_d)
        id_t = const.tile([128, 128], bf16, name="id", tag="id")
        dma(out=id_t, in_=id_d)

        # ---------------- persistent tiles ----------------
        xT = [pers.tile([128, NT], bf16, name=f"xT{k}", tag=f"xT{k}") for k in range(3)]
        qcm = [pers.tile([128, NT], bf16, name=f"q{t}", tag=f"q{t}") for t in range(2)]
        kcm = [pers.tile([128, NT], bf16, name=f"k{t}", tag=f"k{t}") for t in range(2)]
        # v channel-major padded, per (chtile, phase): [128, GUARD+GP+GUARD]
        vcm = [[pers.tile([128, GP + 2 * GUARD], bf16, name=f"vc{c}_{s}", tag=f"vc{c}_{s}")
                for s in range(2)] for c in range(8)]
        vtok = [pers.tile([128, DH], bf16, name=f"vt{s}", tag=f"vt{s}") for s in range(8)]
        Ls = [[pers.tile([128, 392], bf16, name=f"Ls{j}_{s}", tag=f"Ls{j}_{s}") for s in range(2)]
              for j in range(4)]
        a2lo = [[pers.tile([128, 392], bf16, name=f"a2l{j}_{s}", tag=f"a2l{j}_{s}") for s in range(2)]
                for j in range(4)]
        a2hi = [[pers.tile([128, 392], bf16, name=f"a2h{j}_{s}", tag=f"a2h{j}_{s}") for s in range(2)]
                for j in range(4)]
        r_sb = [pers.tile([128, 392], f32, name=f"rsb{s}", tag=f"rsb{s}") for s in range(2)]
        out_cm = [pers.tile([128, NT], bf16, name=f"oc{m}", tag=f"oc{m}") for m in range(3)]

        # zero-init: padded v tiles fully; Ls dead rows
        for c in range(8):
            for s in range(2):
                nc.gpsimd.memset(vcm[c][s], 0.0)
        for j in range(4):
            for s in range(2):
                nc.vector.memset(Ls[j][s][32:64, :], 0.0)
                nc.vector.memset(Ls[j][s][96:128, :], 0.0)

        # ---------------- x load + transpose ----------------
        for tt in range(NTT):
            rows = min(128, NT - tt * 128)
            xt_in = xin.tile([128, DIM], bf16, name="xin", tag="xin")
            dma(out=xt_in[0:rows, :], in_=x_d[tt * 128: tt * 128 + rows, :])
            for kc in range(3):
                tp = ps.tile([128, 512], bf16, name="ps", tag="ps")
                nc.tensor.transpose(tp[0:128, 0:rows],
                                    xt_in[0:rows, kc * 128:(kc + 1) * 128],
                                    id_t[0:rows, 0:rows])
                nc.vector.tensor_copy(xT[kc][:, tt * 128: tt * 128 + rows],
                                      tp[0:128, 0:rows])

        # ---------------- per-group pipeline ----------------
        for g in range(NG):
            sl = g % 2          # phase slot
            c0 = g * 392        # column offset into NT-wide tiles

            # --- QKV channel-major ---
            for mt in range(12 if stage >= 2 else 0):
                qp = ps.tile([128, 512], f32, name="ps", tag="ps")
                for kt in range(3):
                    if mt < 4:
                        w = wqk_t[kt][:, mt * 128:(mt + 1) * 128]
                    else:
                        w = wv_t[kt][:, (mt - 4) * 128:(mt - 3) * 128]
                    nc.tensor.matmul(qp[:, 0:392], w,
                                     xT[kt][:, c0:c0 + 392],
                                     start=(kt == 0), stop=(kt == 2))
                if mt < 4:
                    dst = (qcm if mt < 2 else kcm)[mt % 2]
                    nc.scalar.activation(dst[:, c0:c0 + 392], qp[:, 0:392],
                                         AF.Identity,
                                         bias=bqk_t[:, mt:mt + 1])
                else:
                    ct = mt - 4
                    dst = vcm[ct][sl]
                    dview = dst[:, GUARD: GUARD + 8 * PAD]
                    dview = dview.rearrange("p (i q) -> p i q", q=PAD)
                    dview = dview[:, :, 10:73]
                    dview = dview.rearrange("p i (y x) -> p i y x", x=9)
                    dview = dview[:, :, :, 0:7]
                    sview = qp[:, 0:392].rearrange("p (i y x) -> p i y x",
                                                   y=7, x=7)
                    nc.vector.tensor_copy(dview, sview)

            # --- v token-major (per image pair) ---
            for pr in range(4 if stage >= 3 else 0):
                p = 4 * g + pr
                vp = [ps.tile([128, 512], f32, name="ps", tag="ps") for _ in range(2)]
                for nh in range(2):
                    for pp in range(2):
                        cbase = p * 98 + 49 * pp
                        m = 64 if cbase + 64 <= NT else NT - cbase
                        for kt in range(3):
                            nc.tensor.matmul(
                                vp[nh][64 * pp: 64 * pp + m, :],
                                xT[kt][:, cbase: cbase + m],
                                wv_t[kt][:, nh * 512:(nh + 1) * 512],
                                start=(kt == 0), stop=(kt == 2))
                    eng = nc.vector if nh == 0 else nc.scalar
                    if nh == 0:
                        nc.vector.tensor_copy(
                            vtok[p % 8][0:113, nh * 512:(nh + 1) * 512],
                            vp[nh][0:113, :])
                    else:
                        nc.scalar.activation(
                            vtok[p % 8][0:113, nh * 512:(nh + 1) * 512],
                            vp[nh][0:113, :], AF.Copy)

            # --- depthwise conv (DVE), seeded with B2 via DMA ---
            acc_t = []
            for ct in range(8 if stage >= 4 else 0):
                acc = accp.tile([128, GP], bf16, name=f"acc{ct}", tag=f"acc{ct}")
                dma(out=acc, in_=bass.AP(tensor=b2_d.tensor,
                                         offset=ct * 128 * PAD,
                                         ap=[[PAD, 128], [0, 8], [1, PAD]]))
                for tap in range(9):
                    dy, dx = tap // 3 - 1, tap % 3 - 1
                    delta = 9 * dy + dx
                    src = vcm[ct][sl][:, GUARD + delta: GUARD + delta + GP]
                    nc.vector.scalar_tensor_tensor(
                        out=acc, in0=src,
                        scalar=w9_t[:, ct, tap:tap + 1],
                        in1=acc, op0=ALU.mult, op1=ALU.add)
                acc_t.append(acc)

            # --- qk logits ---
            Lp = [ps.tile([128, 512], f32, name="ps", tag="ps") for _ in range(4)]
            if stage < 5:
                continue
            for ig in range(8):
                i = g * 8 + ig
                for h in range(H):
                    j, hh = h // 2, h % 2
                    t4, row = h // 4, (h % 4) * 32
                    nc.tensor.matmul(
                        Lp[j][64 * hh: 64 * hh + N, ig * N:(ig + 1) * N],
                        kcm[t4][row:row + 32, i * N:(i + 1) * N],
                        qcm[t4][row:row + 32, i * N:(i + 1) * N],
                        start=True, stop=True,
                        tile_position=(row, 64 * hh))
            for j in range(4):
                nc.scalar.activation(Ls[j][sl][0:N, :], Lp[j][0:N, 0:392],
                                     AF.Copy)
                nc.scalar.activation(Ls[j][sl][64:64 + N, :],
                                     Lp[j][64:64 + N, 0:392], AF.Copy)

            # --- talking heads 1 + exp ---
            if stage < 6:
                continue
            E = []
            L2p = [ps.tile([128, 512], f32, name="ps", tag="ps") for _ in range(4)]
            for jo in range(4):
                for ji in range(4):
                    nc.tensor.matmul(L2p[jo][:, 0:392],
                                     w1_t[:, jo * 4 + ji, :],
                                     Ls[ji][sl],
                                     start=(ji == 0), stop=(ji == 3))
            for jo in range(4):
                e = mid.tile([128, 392], bf16, name="E", tag="E")
                nc.scalar.activation(e, L2p[jo][:, 0:392], AF.Exp)
                nc.vector.tensor_mul(e, e, expb_t[jo])
                E.append(e)

            # --- softmax denominator ---
            csp = ps.tile([128, 512], f32, name="ps", tag="ps")
            for j in range(4):
                nc.tensor.matmul(csp[32 * j: 32 * j + 2, 0:392], sel_t, E[j],
                                 start=True, stop=True,
                                 tile_position=(0, 32 * j))
            for j in range(4):
                nc.vector.reciprocal(r_sb[sl][32 * j: 32 * j + 2, :],
                                     csp[32 * j: 32 * j + 2, 0:392])

            # --- normalize + talking heads 2 ---
            A = []
            for j in range(4):
                rp = ps.tile([128, 512], f32, name="ps", tag="ps")
                nc.tensor.matmul(rp[:, 0:392], dlt_t[32 * j: 32 * j + 2, :],
                                 r_sb[sl][32 * j: 32 * j + 2, :],
                                 start=True, stop=True,
                                 tile_position=(32 * j, 0))
                a = mid.tile([128, 392], bf16, name="A", tag="A")
                nc.vector.tensor_mul(a, E[j], rp[:, 0:392])
                A.append(a)
            A2p = [ps.tile([128, 512], f32, name="ps", tag="ps") for _ in range(4)]
            for jo in range(4):
                for ji in range(4):
                    nc.tensor.matmul(A2p[jo][:, 0:392],
                                     w2_t[:, jo * 4 + ji, :],
                                     A[ji],
                                     start=(ji == 0), stop=(ji == 3))
            for jo in range(4):
                nc.scalar.activation(a2lo[jo][sl][0:113, :],
                                     A2p[jo][0:113, 0:392], AF.Copy)
                nc.vector.tensor_copy(a2hi[jo][sl][0:N, :],
                                      A2p[jo][64:64 + N, 0:392])
                nc.vector.tensor_copy(a2hi[jo][sl][64:64 + N, :],
                                      A2p[jo][0:N, 0:392])

            # --- attention * V, assembly, relu ---
            if stage < 7:
                continue
            relu_t = []
            for ct in range(8):
                # parity-split psum tiles: even/odd images use different PE
                # row-tile positions (0 vs 64); giving each its own psum bank
                # avoids concurrent same-bank writes from independent tiles.
                op2 = [ps.tile([128, 512], f32, name="ps", tag="ps")
                       for _ in range(2)]
                jo, hh = ct // 2, ct % 2
                for ig in range(8):
                    i = g * 8 + ig
                    pp = i % 2
                    a2 = (a2lo if hh == pp else a2hi)[jo][sl]
                    nc.tensor.matmul(
                        op2[pp][:, (ig // 2) * N:(ig // 2 + 1) * N],
                        vtok[(i // 2) % 8][64 * pp: 64 * pp + N,
                                           ct * 128:(ct + 1) * 128],
                        a2[64 * pp: 64 * pp + N, ig * N:(ig + 1) * N],
                        start=True, stop=True)
                if stage < 8:
                    continue
                tmp = mid.tile([128, 392], bf16, name="tmp", tag="tmp", bufs=3)
                accv = acc_t[ct].rearrange("p (i4 two q) -> p i4 two q",
                                           two=2, q=PAD)
                tmpv = tmp.rearrange("p (i4 two y x) -> p i4 two y x",
                                     two=2, y=7, x=7)
                for pp in range(2):
                    vlv = accv[:, :, pp, 10:73]
                    vlv = vlv.rearrange("p i (y x) -> p i y x", x=9)
                    vlv = vlv[:, :, :, 0:7]
                    nc.vector.tensor_add(
                        tmpv[:, :, pp],
                        op2[pp][:, 0:196].rearrange("p (i y x) -> p i y x",
                                                    y=7, x=7),
                        vlv)
                rl = mid.tile([128, 392], bf16, name="rl", tag="rl", bufs=10)
                nc.scalar.activation(rl, tmp, AF.Relu)
                relu_t.append(rl)

            # --- projection ---
            if stage < 9:
                continue
            for mt in range(3):
                pp_ = ps.tile([128, 512], f32, name="ps", tag="ps")
                for kt in range(8):
                    nc.tensor.matmul(pp_[:, 0:392],
                                     wp_t[kt][:, mt * 128:(mt + 1) * 128],
                                     relu_t[kt],
                                     start=(kt == 0), stop=(kt == 7))
                nc.scalar.activation(out_cm[mt][:, c0:c0 + 392],
                                     pp_[:, 0:392], AF.Identity,
                                     bias=bp_t[:, mt:mt + 1])

        # ---------------- output transpose + store ----------------
        if stage < 10:
            z0 = stg.tile([128, DIM], f32, name="z0", tag="st")
            nc.vector.memset(z0, 0.0)
            for tt in range(NTT):
                rows = min(128, NT - tt * 128)
                nc.sync.dma_start(out=out_d[tt * 128: tt * 128 + rows, :],
                                  in_=z0[0:rows, :])
        for tt in range(NTT if stage >= 10 else 0):
            rows = min(128, NT - tt * 128)
            st = stg.tile([128, DIM], f32, name="st", tag="st")
            for mt in range(3):
                tp = ps.tile([128, 512], bf16, name="ps", tag="ps")
                nc.tensor.transpose(tp[0:rows, 0:128],
                                    out_cm[mt][:, tt * 128: tt * 128 + rows],
                                    id_t[0:128, 0:128])
                if mt == 1:
                    nc.vector.tensor_copy(st[0:rows, mt * 128:(mt + 1) * 128],
                                          tp[0:rows, 0:128])
                else:
                    nc.scalar.activation(st[0:rows, mt * 128:(mt + 1) * 128],
                                         tp[0:rows, 0:128], AF.Copy)
            dma(out=out_d[tt * 128: tt * 128 + rows, :], in_=st[0:rows, :])

    nc.compile()
    return nc


_CACHE = {}


def _get_program(n_imgs):
    if n_imgs not in _CACHE:
        _CACHE[n_imgs] = build_program(n_imgs)
    return _CACHE[n_imgs]


def make_in_maps(inputs, n_cores=NCORES):
    """Host prep: shard x, build replicated constants."""
    consts = _host_consts(inputs)
    x = np.asarray(inputs['x'], np.float32)
    B = x.shape[0]
    ni = B // n_cores
    x = x.reshape(B, N, DIM)
    in_maps = []
    for c in range(n_cores):
        m = dict(consts)
        m['x'] = x[c * ni:(c + 1) * ni].reshape(ni * N, DIM).astype(_BF16)
        in_maps.append(m)
    return in_maps, ni


def kernel(**inputs):
    from concourse import bass_utils
    in_maps, ni = make_in_maps(inputs)
    nc = _get_program(ni)
    res = bass_utils.run_bass_kernel_spmd(
        nc, in_maps, core_ids=list(range(NCORES)))
    B = np.asarray(inputs['x']).shape[0]
    out = np.concatenate([r['out'] for r in res.results], axis=0)
    return out.reshape(B, R, R, DIM).astype(np.float32)

